# revision 80
# baseline (speedup 1.0000x reference)
"""Causal self-attention (B=8, T=1024, C=2048, H=16) on 8 TRN2 NeuronCores.

Strategy: data-parallel over batch — core i computes the full attention block
for batch element i (weights replicated, no collectives).

Key optimizations (667us -> 591us -> ~534us at nominal clock):
  - x is transposed and cast to bf16 on the HOST: xT tiles DMA straight into
    SBUF, eliminating the on-device transpose phase (128 PE transposes +
    128 DVE copies + staging buffers, ~40us of low-PE-occupancy time)
  - B1's input feed is balanced across the three DMA-capable queues
    (sync/scalar/gpsimd) in consumption order with 0.25-2MB transfers
    (small DMAs only sustain ~100GB/s per queue; big ones ~370GB/s), so
    the first matmul issues ~11.5us in and B1 streams without stalls
  - head-0 k-weights preload into a persistent pool during B1: the normal
    per-head weight staging reuses the w_v SBUF space, and the pool-close
    barrier would gate that DMA on the LAST B1 matmul
  - B1's last pass runs t-major so its PSUM->SBUF bias-adds trail the last
    matmul by one tile instead of eight (same pool-close barrier)
  - per-head pipelining: each head's attention (softmax on ACT/DVE) is
    emitted interleaved with the NEXT head's q/k projection matmuls
  - fine-causal S/exp: S matmuls and exp cover only live columns
  - softmax denominators: bf16 tree-adds on the DVE, then cross-partition
    sums on the otherwise-idle gpsimd (partition_all_reduce) in the head
    loop; the latency-critical drain uses PE ones-matmuls instead
  - drain of the last head interleaves with phase-D column 0 (t=0..2) in
    the freed psB PSUM banks; the output bias for those tiles is folded
    into PSUM via a (1/128)-matmul against the broadcast bias so the final
    PSUM->SBUF drain is a plain ACT copy off the DVE critical path
  - phase D: w_proj column 0 preloaded during the last 4 heads (gpsimd);
    columns 1-3 stream in halves on queue pairs at phase-D start; all
    columns run t-major so bias-adds + stores spread out and PSUM banks
    recycle without column-boundary stalls; the final tile's bias+store is
    split in half across queues to start the end-of-kernel drain sooner

Per-core pipeline (Tile framework, all matmuls bf16 on the PE):
  B1) v = x @ Wv + bv in natural layout (8 PSUM banks, one per t-tile),
      consuming the xT / wv streams as they land
  B2+C) per head: k/q chunk matmuls (W-stationary, xT moving, bias on ACT)
     interleaved with the previous head's S^T = kT^T qT, exp on ACT,
     diagonal-triangle mask, denominator tree, PV accumulation, and the
     1/denom multiply -> attnT bf16
  D) y = attnT-stationary @ w_proj (moving, bf16) + bias, output f32.

Note: the chip occasionally runs the whole NEFF at a degraded 2.0GHz PE
clock (~640us instead of ~534us); this is a device power state, not a
property of the kernel schedule.
"""

import sys

if "/opt/trn_rl_repo" not in sys.path:
    sys.path.insert(0, "/opt/trn_rl_repo")

import numpy as np
import ml_dtypes

import concourse.bass as bass
import concourse.bass_isa as bass_isa
import concourse.mybir as mybir
import concourse.tile as tile
from concourse import bacc
from concourse.bass_utils import run_bass_kernel_spmd

B, T, C = 8, 1024, 2048
H, HD = 16, 128
N_CORES = 8
P = 128            # partition dim
TQ = 512           # moving-operand tile (q positions per matmul)
KK = C // P        # 16 contraction tiles over C
TT = T // P        # 8 tiles over T
NQ = T // TQ       # 2 q-tiles
NCT = C // TQ      # 4 column tiles over C
SCALE = 1.0 / float(np.sqrt(HD))

f32 = mybir.dt.float32
bf16 = mybir.dt.bfloat16
AFT = mybir.ActivationFunctionType

_NC_CACHE = None


def build_nc():
    nc = bacc.Bacc("TRN2", target_bir_lowering=False, debug=False,
                   num_devices=N_CORES)

    # x pre-transposed on host: xT_pm[p, kk, t] = x[t, kk*128+p], bf16
    xT_d = nc.declare_dram_parameter("xT_pm", [P, KK, T], bf16, isOutput=False)
    # q/k weight chunks, partition-major: wqk[p, m, kk, n] =
    # w_attn[kk*128+p, m*128+n] for m < 32 (q columns pre-scaled)
    wqk = nc.declare_dram_parameter("wqk_pm", [P, 2 * KK, KK, P], bf16,
                                    isOutput=False)
    bqk = nc.declare_dram_parameter("bqk_pm", [P, 2 * KK], f32, isOutput=False)
    # v / proj weights in column-chunk-major layout:
    #   wvc[ct, p, kk, c] = w_attn[kk*128+p, 2C + ct*512 + c]
    #   wpc[ct, p, kk, c] = w_proj[kk*128+p, ct*512 + c]
    wvc = nc.declare_dram_parameter("wv_cm", [NCT, P, KK, TQ], bf16,
                                    isOutput=False)
    wpc = nc.declare_dram_parameter("wp_cm", [NCT, P, KK, TQ], bf16,
                                    isOutput=False)
    bv = nc.declare_dram_parameter("bv_bc", [P, C], bf16, isOutput=False)
    bp = nc.declare_dram_parameter("bp_bc", [P, C], bf16, isOutput=False)
    tri_d = nc.declare_dram_parameter("tri", [P, P], bf16, isOutput=False)
    ones_b = nc.declare_dram_parameter("ones_b", [P, P], bf16, isOutput=False)
    oinv_b = nc.declare_dram_parameter("oinv_b", [P, P], bf16, isOutput=False)
    y = nc.declare_dram_parameter("y", [T, C], f32, isOutput=True)

    with tile.TileContext(nc) as tc:
        with tc.tile_pool(name="consts", bufs=1) as consts, \
             tc.tile_pool(name="resid", bufs=1) as resid, \
             tc.tile_pool(name="wp0", bufs=1) as wp0p, \
             tc.tile_pool(name="qkw0", bufs=1) as qkw0, \
             tc.tile_pool(name="ybuf", bufs=4) as ybuf:

            # ---- constants (DMAs issued inside the B1 feed below, ordered
            # by first use) ----
            bv_sb = consts.tile([P, C], bf16, tag="bv", name="bv")
            tri_sb = consts.tile([P, P], bf16, tag="tri", name="tri")
            ones_sb = consts.tile([P, P], bf16, tag="ones", name="ones")
            oinv_sb = consts.tile([P, P], bf16, tag="oinv", name="oinv")
            bqk_sb = consts.tile([P, 2 * KK], f32, tag="bqk", name="bqk")
            bp_sb = consts.tile([P, C], bf16, tag="bp", name="bp")

            # w_proj column 0, preloaded during the tail of B2+C
            wp0_sb = wp0p.tile([P, KK, TQ], bf16, tag="wp0", name="wp0")
            # head-0 q/k weights, preloaded during B1 on gpsimd so the first
            # B2+C matmul is not gated on a DMA that (because its SBUF space
            # is recycled from the w_v tiles) can only start after the last
            # B1 matmul finishes
            wtk0_sb = qkw0.tile([P, KK, P], bf16, tag="wtk0", name="wtk0")

            # ---- persistent intermediates (bf16) ----
            v = [resid.tile([P, C], bf16, tag=f"v{i}", name=f"v{i}") for i in range(TT)]
            attnT = [resid.tile([P, T], bf16, tag=f"attnT{i}", name=f"attnT{i}")
                     for i in range(H)]

            with tc.tile_pool(name="xT", bufs=1) as xTp:
                xT_sb = xTp.tile([P, KK, T], bf16, tag="xT", name="xT")
                # B1 feed balanced over the three DMA-capable queues
                # (~120GB/s per queue when all active; the DMA fabric
                # services queues roughly fairly, so each queue leads with
                # the earliest-consumed tiles):
                #   sync:   xT kk 0-7 in pairs, then wv col1 lo half
                #   scalar: wv col0 groups, then wv col1 hi half
                #   gpsimd: xT kk 8-15 in pairs, then bv, wv col2/3,
                #           consts, head-0 weights
                for lo, hi in ((0, 2), (2, 4), (4, 6), (6, 8)):
                    nc.sync.dma_start(out=xT_sb[:, lo:hi, :],
                                      in_=xT_d[:, lo:hi, :])
                for lo, hi in ((8, 10), (10, 12), (12, 14), (14, 16)):
                    nc.gpsimd.dma_start(out=xT_sb[:, lo:hi, :],
                                        in_=xT_d[:, lo:hi, :])

                with tc.tile_pool(name="wvp", bufs=1) as wvp:
                    wv_sb = [wvp.tile([P, KK, TQ], bf16, tag=f"wv{ct}",
                                      name=f"wv{ct}") for ct in range(NCT)]
                    for lo, hi in ((0, 2), (2, 4), (4, 8), (8, 12), (12, 16)):
                        nc.scalar.dma_start(out=wv_sb[0][:, lo:hi, :],
                                            in_=wvc[0, :, lo:hi, :])
                    nc.sync.dma_start(out=wv_sb[1][:, 0:8, :],
                                      in_=wvc[1, :, 0:8, :])
                    nc.scalar.dma_start(out=wv_sb[1][:, 8:16, :],
                                        in_=wvc[1, :, 8:16, :])
                    nc.gpsimd.dma_start(out=bv_sb, in_=bv[:])
                    nc.gpsimd.dma_start(out=wv_sb[2], in_=wvc[2])
                    nc.gpsimd.dma_start(out=wv_sb[3], in_=wvc[3])
                    # remaining consts + head-0 weights (gpsimd, after wv)
                    nc.gpsimd.dma_start(out=tri_sb, in_=tri_d[:])
                    nc.gpsimd.dma_start(out=ones_sb, in_=ones_b[:])
                    nc.gpsimd.dma_start(out=oinv_sb, in_=oinv_b[:])
                    nc.gpsimd.dma_start(out=bqk_sb, in_=bqk[:])
                    nc.gpsimd.dma_start(out=bp_sb, in_=bp[:])
                    nc.gpsimd.dma_start(out=wtk0_sb, in_=wqk[:, KK, :, :])

                    # ---- Phase B1: v = x @ Wv + bv, natural layout ----
                    # Pass 0 runs kk-major (matching the DMA arrival order of
                    # the streamed wv column-0 tiles); passes 1-3 run t-major
                    # so each psV bank closes early and its bias-add overlaps
                    # the remaining matmuls instead of bursting serialized on
                    # the DVE at the pass boundary (which gates the next
                    # pass's — and ultimately B2C's — bank reuse).
                    with tc.tile_pool(name="psV", bufs=8, space=bass.MemorySpace.PSUM) as psVp:
                        for ct in range(NCT):
                            psV = [psVp.tile([P, TQ], f32, tag="psV", name="psV")
                                   for _ in range(TT)]
                            if ct == 0:
                                for kk in range(KK):
                                    wvt = wv_sb[ct][:, kk, :]
                                    for t in range(TT):
                                        nc.tensor.matmul(
                                            psV[t],
                                            xT_sb[:, kk, t * P:(t + 1) * P],
                                            wvt,
                                            start=(kk == 0),
                                            stop=(kk == KK - 1))
                                for t in range(TT):
                                    nc.vector.tensor_add(
                                        v[t][:, ct * TQ:(ct + 1) * TQ], psV[t],
                                        bv_sb[:, ct * TQ:(ct + 1) * TQ])
                            else:
                                for t in range(TT):
                                    for kk in range(KK):
                                        nc.tensor.matmul(
                                            psV[t],
                                            xT_sb[:, kk, t * P:(t + 1) * P],
                                            wv_sb[ct][:, kk, :],
                                            start=(kk == 0),
                                            stop=(kk == KK - 1))
                                    nc.vector.tensor_add(
                                        v[t][:, ct * TQ:(ct + 1) * TQ],
                                        psV[t],
                                        bv_sb[:, ct * TQ:(ct + 1) * TQ])

                # ---- Merged phase B2+C: per head, the q/k projection
                # chunks (pure PE work) are interleaved with the PREVIOUS
                # head's attention so the softmax's ACT/DVE work and its
                # cross-engine latency hide completely under the projection
                # matmuls.
                #
                # Attention is fine-causal: S matmuls and exp cover only live
                # columns [kt*128, T); the permanently-masked columns of each
                # eS tile are zeroed ONCE and never written again, so the
                # denominator tree-adds can read full 512-wide slices. The
                # diagonal 128x128 triangle is zeroed by a 0/1 mask multiply.
                # PSUM: psB 3 + psS 3 + psO 2 = 8 banks; a matmul with
                # start=True clears its whole bank, so every accumulation
                # group owns a full bank.
                with tc.tile_pool(name="qkp", bufs=2) as qkp, \
                     tc.tile_pool(name="eSp", bufs=1) as eSp, \
                     tc.tile_pool(name="dsc", bufs=2) as dsc, \
                     tc.tile_pool(name="ctmp", bufs=2) as ctmp, \
                     tc.tile_pool(name="wst", bufs=2) as wst, \
                     tc.tile_pool(name="psB", bufs=3, space=bass.MemorySpace.PSUM) as psB, \
                     tc.tile_pool(name="psS", bufs=3, space=bass.MemorySpace.PSUM) as psS, \
                     tc.tile_pool(name="psO", bufs=2, space=bass.MemorySpace.PSUM) as psO:
                    eSab = [[eSp.tile([P, T], bf16, tag=f"eS{s}_{kt}",
                                      name=f"eS{s}_{kt}")
                             for kt in range(TT)] for s in range(2)]
                    for s in range(2):
                        for kt in range(1, TT):
                            nc.vector.memset(eSab[s][kt][:, 0:kt * P], 0)

                    tri = tri_sb  # [128,128] q>=k triangle
                    st = [dict() for _ in range(H)]

                    def emit_S(ph, kt):
                        pool, ptag = psS, "psS"
                        q0 = kt * P
                        eS = eSab[ph % 2]
                        kblk = st[ph]["kT"][:, kt * P:(kt + 1) * P]
                        qTt = st[ph]["qT"]
                        if kt < 4:
                            pa = pool.tile([P, TQ], f32, tag=ptag, name="pssa")
                            nc.tensor.matmul(pa[:, q0:TQ], kblk,
                                             qTt[:, q0:TQ],
                                             start=True, stop=True)
                            pb = pool.tile([P, TQ], f32, tag=ptag, name="pssb")
                            nc.tensor.matmul(pb, kblk, qTt[:, TQ:T],
                                             start=True, stop=True)
                            nc.scalar.activation(out=eS[kt][:, q0:TQ],
                                                 in_=pa[:, q0:TQ],
                                                 func=AFT.Exp)
                            nc.scalar.activation(out=eS[kt][:, TQ:T], in_=pb,
                                                 func=AFT.Exp)
                        else:
                            pb = pool.tile([P, TQ], f32, tag=ptag, name="pssb")
                            nc.tensor.matmul(pb[:, q0 - TQ:TQ], kblk,
                                             qTt[:, q0:T],
                                             start=True, stop=True)
                            nc.scalar.activation(out=eS[kt][:, q0:T],
                                                 in_=pb[:, q0 - TQ:TQ],
                                                 func=AFT.Exp)
                        nc.vector.tensor_mul(eS[kt][:, q0:q0 + P],
                                             eS[kt][:, q0:q0 + P], tri)

                    def emit_tree_qt1(ph):
                        eS = eSab[ph % 2]
                        t1 = dsc.tile([P, TQ], bf16, tag="t1", name="t1")
                        t2 = dsc.tile([P, TQ], bf16, tag="t2", name="t2")
                        t3 = dsc.tile([P, TQ], bf16, tag="t3", name="t3")
                        t4 = dsc.tile([P, TQ], bf16, tag="t4", name="t4")
                        nc.vector.tensor_add(t1, eS[0][:, TQ:T], eS[1][:, TQ:T])
                        nc.vector.tensor_add(t2, eS[2][:, TQ:T], eS[3][:, TQ:T])
                        nc.vector.tensor_add(t3, eS[4][:, TQ:T], eS[5][:, TQ:T])
                        nc.vector.tensor_add(t4, eS[6][:, TQ:T], eS[7][:, TQ:T])
                        nc.vector.tensor_add(t1, t1, t2)
                        nc.vector.tensor_add(t3, t3, t4)
                        nc.vector.tensor_add(t1, t1, t3)
                        st[ph]["t1"] = t1

                    def emit_tree_qt0(ph):
                        eS = eSab[ph % 2]
                        u1 = dsc.tile([P, TQ], bf16, tag="t5", name="u1")
                        u2 = dsc.tile([P, TQ], bf16, tag="t6", name="u2")
                        nc.vector.tensor_add(u1, eS[0][:, 0:TQ], eS[1][:, 0:TQ])
                        nc.vector.tensor_add(u2, eS[2][:, 0:TQ], eS[3][:, 0:TQ])
                        nc.vector.tensor_add(u1, u1, u2)
                        st[ph]["u1"] = u1

                    def emit_denoms(ph, on_pe=False):
                        # In the pipelined head loop the cross-partition sums
                        # run on the (idle) gpsimd engine — frees the PE of
                        # 2x512 ones-matmul columns per head. gpsimd takes
                        # ~3.6us per reduce though, so the latency-critical
                        # DRAIN uses the PE ones-matmul (0.2us each) instead.
                        if on_pe:
                            psd1 = psS.tile([P, TQ], f32, tag="psS", name="psd1")
                            nc.tensor.matmul(psd1, ones_sb, st[ph]["t1"],
                                             start=True, stop=True)
                            psd0 = psS.tile([P, TQ], f32, tag="psS", name="psd0")
                            nc.tensor.matmul(psd0, ones_sb, st[ph]["u1"],
                                             start=True, stop=True)
                            st[ph]["psd0"], st[ph]["psd1"] = psd0, psd1
                            return
                        d1 = ctmp.tile([P, TQ], f32, tag="d1", name="d1")
                        nc.gpsimd.partition_all_reduce(
                            d1, st[ph]["t1"], channels=P,
                            reduce_op=bass_isa.ReduceOp.add)
                        d0 = ctmp.tile([P, TQ], f32, tag="d0", name="d0")
                        nc.gpsimd.partition_all_reduce(
                            d0, st[ph]["u1"], channels=P,
                            reduce_op=bass_isa.ReduceOp.add)
                        st[ph]["psd0"], st[ph]["psd1"] = d0, d1

                    def emit_PV(ph, kt):
                        eS = eSab[ph % 2]
                        if kt == 0:
                            st[ph]["pso0"] = psO.tile([P, TQ], f32, tag="psO",
                                                      name="pso0")
                            st[ph]["pso1"] = psO.tile([P, TQ], f32, tag="psO",
                                                      name="pso1")
                        lhsT = v[kt][:, ph * P:(ph + 1) * P]
                        # start=True must span the whole bank (it clears it);
                        # accumulating matmuls shrink to the live columns
                        if kt == 0:
                            nc.tensor.matmul(
                                st[ph]["pso0"], lhsT, eS[0][:, 0:TQ],
                                start=True, stop=False)
                            nc.tensor.matmul(
                                st[ph]["pso1"], lhsT, eS[0][:, TQ:T],
                                start=True, stop=False)
                        else:
                            q0 = kt * P
                            if kt < 4:
                                nc.tensor.matmul(
                                    st[ph]["pso0"][:, q0:TQ], lhsT,
                                    eS[kt][:, q0:TQ],
                                    start=False, stop=(kt == 3))
                                nc.tensor.matmul(
                                    st[ph]["pso1"], lhsT, eS[kt][:, TQ:T],
                                    start=False, stop=False)
                            else:
                                lo = max(q0, TQ)
                                nc.tensor.matmul(
                                    st[ph]["pso1"][:, lo - TQ:TQ], lhsT,
                                    eS[kt][:, lo:T],
                                    start=False, stop=(kt == TT - 1))

                    def emit_div(ph):
                        # ~18-bit reciprocal; denominators in [1, ~2e5]
                        rec1 = ctmp.tile([P, TQ], f32, tag="rec", name="rec1")
                        nc.vector.reciprocal_approx_fast(out=rec1,
                                                         in_=st[ph]["psd1"])
                        nc.vector.tensor_mul(attnT[ph][:, TQ:T],
                                             st[ph]["pso1"], rec1)
                        rec0 = ctmp.tile([P, TQ], f32, tag="rec", name="rec0")
                        nc.vector.reciprocal_approx_fast(out=rec0,
                                                         in_=st[ph]["psd0"])
                        nc.vector.tensor_mul(attnT[ph][:, 0:TQ],
                                             st[ph]["pso0"], rec0)
                        st[ph].clear()

                    for i in range(H):
                        ph = i - 1
                        # --- k chunk of head i ---
                        # wqk DMAs issue from the (idle) sync queue — the
                        # scalar sequencer is busy with exps/identities
                        if i == 0:
                            wtk = wtk0_sb
                        else:
                            wtk = wst.tile([P, KK, P], bf16, tag="wt", name="wtk")
                            nc.sync.dma_start(out=wtk, in_=wqk[:, KK + i, :, :])
                        kTt = qkp.tile([P, T], bf16, tag="kT", name="kTt")
                        psk = [psB.tile([P, TQ], f32, tag="psB", name="psB")
                               for _ in range(NQ)]
                        for kk in range(KK):
                            for qt in range(NQ):
                                nc.tensor.matmul(
                                    psk[qt], wtk[:, kk, :],
                                    xT_sb[:, kk, qt * TQ:(qt + 1) * TQ],
                                    start=(kk == 0), stop=(kk == KK - 1))
                            if ph >= 0 and kk < TT:
                                emit_S(ph, kk)
                        for qt in range(NQ):
                            nc.scalar.activation(
                                out=kTt[:, qt * TQ:(qt + 1) * TQ], in_=psk[qt],
                                func=AFT.Identity,
                                bias=bqk_sb[:, KK + i:KK + i + 1])
                        # --- q chunk of head i ---
                        wtq = wst.tile([P, KK, P], bf16, tag="wt", name="wtq")
                        nc.gpsimd.dma_start(out=wtq, in_=wqk[:, i, :, :])
                        # preload w_proj column 0 during the last 4 heads
                        # (gpsimd queue is otherwise idle by then)
                        if i >= H - 4:
                            q4 = i - (H - 4)
                            nc.gpsimd.dma_start(
                                out=wp0_sb[:, q4 * 4:(q4 + 1) * 4, :],
                                in_=wpc[0, :, q4 * 4:(q4 + 1) * 4, :])
                        qTt = qkp.tile([P, T], bf16, tag="qT", name="qTt")
                        psq = [psB.tile([P, TQ], f32, tag="psB", name="psB")
                               for _ in range(NQ)]
                        for kk in range(KK):
                            for qt in range(NQ):
                                nc.tensor.matmul(
                                    psq[qt], wtq[:, kk, :],
                                    xT_sb[:, kk, qt * TQ:(qt + 1) * TQ],
                                    start=(kk == 0), stop=(kk == KK - 1))
                            if ph >= 0:
                                if kk == 0:
                                    emit_tree_qt1(ph)
                                elif kk == 2:
                                    emit_tree_qt0(ph)
                                elif kk == 3:
                                    emit_denoms(ph)
                                if kk % 2 == 1:
                                    emit_PV(ph, kk // 2)
                        for qt in range(NQ):
                            nc.scalar.activation(
                                out=qTt[:, qt * TQ:(qt + 1) * TQ], in_=psq[qt],
                                func=AFT.Identity, bias=bqk_sb[:, i:i + 1])
                        if ph >= 0:
                            emit_div(ph)
                        st[i]["kT"], st[i]["qT"] = kTt, qTt
                    # --- drain: attention of the last head, interleaved with
                    # phase-D column 0, t-tiles 0..2 (using the 3 freed psB
                    # banks + the preloaded wp0) so the PE never idles while
                    # the softmax's ACT/DVE chain runs ---
                    ph = H - 1
                    for kt in range(TT):
                        emit_S(ph, kt)
                    psD = [psB.tile([P, TQ], f32, tag="psB", name=f"psD{t}")
                           for t in range(3)]

                    def dcol0(lo, hi):
                        for kk in range(lo, hi):
                            wpt = wp0_sb[:, kk, :]
                            for t in range(3):
                                nc.tensor.matmul(
                                    psD[t], attnT[kk][:, t * P:(t + 1) * P],
                                    wpt, start=(kk == 0), stop=False)

                    dcol0(0, 5)
                    # fold the output bias into the PSUM accumulation
                    # (oinv^T @ bp_bc = broadcast bias row) so the final
                    # PSUM->SBUF drain is a plain copy the ACT engine can do
                    for t in range(3):
                        nc.tensor.matmul(psD[t], oinv_sb, bp_sb[:, 0:TQ],
                                         start=False, stop=False)
                    emit_tree_qt1(ph)
                    emit_tree_qt0(ph)
                    dcol0(5, 9)
                    emit_denoms(ph, on_pe=True)
                    # reciprocals early (they only need the denominators) so
                    # the post-PV critical chain is just the two multiplies;
                    # the [TQ:T] half is not needed by the kk=15 psD matmuls
                    # so it runs on gpsimd off the critical path
                    rec1 = ctmp.tile([P, TQ], f32, tag="rec", name="rec1d")
                    nc.vector.reciprocal_approx_fast(out=rec1,
                                                     in_=st[ph]["psd1"])
                    rec0 = ctmp.tile([P, TQ], f32, tag="rec", name="rec0d")
                    nc.vector.reciprocal_approx_fast(out=rec0,
                                                     in_=st[ph]["psd0"])
                    for kt in range(TT):
                        emit_PV(ph, kt)
                    dcol0(9, 15)
                    # [0:TQ] first — the kk=15 psD matmuls only need that half
                    nc.vector.tensor_mul(attnT[ph][:, 0:TQ],
                                         st[ph]["pso0"], rec0)
                    nc.vector.tensor_mul(attnT[ph][:, TQ:T],
                                         st[ph]["pso1"], rec1)
                    st[ph].clear()
                    for t in range(3):
                        nc.tensor.matmul(
                            psD[t], attnT[H - 1][:, t * P:(t + 1) * P],
                            wp0_sb[:, KK - 1, :], start=False, stop=True)
                    # drain the three psD banks on two engines in parallel
                    # (plain copies — the bias is already in PSUM); the B2C
                    # pool close, and with it phase D, waits on the last one
                    for t in range(3):
                        y_sb = ybuf.tile([P, TQ], f32, tag="y_sb", name="y_sb")
                        if t == 1:
                            nc.vector.tensor_copy(y_sb, psD[t])
                        else:
                            nc.scalar.activation(out=y_sb, in_=psD[t],
                                                 func=AFT.Identity)
                        deng = nc.sync if t % 2 == 0 else nc.scalar
                        deng.dma_start(out=y[t * P:(t + 1) * P, 0:TQ],
                                       in_=y_sb)

            # ---- Phase D (rest): output projection ----
            # Column 0 t=0..2 already computed during the drain. Columns 1-3
            # stream as single 2MB DMAs issued up-front on three different
            # queues so they always arrive ahead of consumption. The last
            # column runs t-major so the final bias-adds + stores spread
            # across its whole span instead of bunching at the end.
            with tc.tile_pool(name="wps", bufs=1) as wps:
                  with tc.tile_pool(name="psY", bufs=8, space=bass.MemorySpace.PSUM) as psYp:
                      wp_sb = [wp0_sb] + [
                          wps.tile([P, KK, TQ], bf16, tag=f"wp{ct}",
                                   name=f"wp{ct}") for ct in range(1, NCT)]
                      # each column split across two queues so col1 lands
                      # before its first matmul needs it (columns are
                      # consumed ~14us apart; a full 6MB three-way race
                      # delivers col1 too late)
                      for ct, engs in ((1, (nc.gpsimd, nc.sync)),
                                       (2, (nc.sync, nc.scalar)),
                                       (3, (nc.gpsimd, nc.scalar))):
                          engs[0].dma_start(out=wp_sb[ct][:, 0:8, :],
                                            in_=wpc[ct, :, 0:8, :])
                          engs[1].dma_start(out=wp_sb[ct][:, 8:16, :],
                                            in_=wpc[ct, :, 8:16, :])
                      # Never-used dummy allocs: skip the PSUM banks still
                      # owned by the drain's psD tiles (their bias-adds are
                      # serialized on the DVE) so the first matmuls land on
                      # banks the exps freed earlier.
                      for _ in range(3):
                          psYp.tile([P, TQ], f32, tag="psY", name="psYdummy")
                      # all columns t-major: each t-tile's accumulation
                      # closes early, so its bias-add + store overlap the
                      # next t-tile's matmuls instead of bursting at the
                      # column end and stalling the next column's banks
                      for ct in range(NCT):
                          ts = range(3, TT) if ct == 0 else range(TT)
                          for t in ts:
                              psY = psYp.tile([P, TQ], f32, tag="psY",
                                              name="psY")
                              for kk in range(KK):
                                  nc.tensor.matmul(
                                      psY, attnT[kk][:, t * P:(t + 1) * P],
                                      wp_sb[ct][:, kk, :],
                                      start=(kk == 0), stop=(kk == KK - 1))
                              y_sb = ybuf.tile([P, TQ], f32, tag="y_sb",
                                               name="y_sb")
                              c0 = ct * TQ
                              if ct == NCT - 1 and t == TT - 1:
                                  # final tile: bias+store in halves so the
                                  # kernel-end drain starts sooner
                                  h = TQ // 2
                                  nc.vector.tensor_add(
                                      y_sb[:, 0:h], psY[:, 0:h],
                                      bp_sb[:, c0:c0 + h])
                                  nc.sync.dma_start(
                                      out=y[t * P:(t + 1) * P, c0:c0 + h],
                                      in_=y_sb[:, 0:h])
                                  nc.vector.tensor_add(
                                      y_sb[:, h:TQ], psY[:, h:TQ],
                                      bp_sb[:, c0 + h:c0 + TQ])
                                  nc.scalar.dma_start(
                                      out=y[t * P:(t + 1) * P,
                                            c0 + h:c0 + TQ],
                                      in_=y_sb[:, h:TQ])
                              else:
                                  nc.vector.tensor_add(
                                      y_sb, psY, bp_sb[:, c0:c0 + TQ])
                                  deng = nc.sync if t % 2 == 0 else nc.scalar
                                  deng.dma_start(
                                      out=y[t * P:(t + 1) * P, c0:c0 + TQ],
                                      in_=y_sb)

    nc.compile()
    return nc


def _get_nc():
    global _NC_CACHE
    if _NC_CACHE is None:
        _NC_CACHE = build_nc()
    return _NC_CACHE


def make_in_maps(inputs):
    x = np.asarray(inputs["x"], dtype=np.float32)
    w_attn = np.asarray(inputs["w_attn"], dtype=np.float32)
    b_attn = np.asarray(inputs["b_attn"], dtype=np.float32)
    w_proj = np.asarray(inputs["w_proj"], dtype=np.float32)
    b_proj = np.asarray(inputs["b_proj"], dtype=np.float32)

    bf = ml_dtypes.bfloat16

    # q/k weights, scale folded into q: [P, 32, KK, P] partition-major
    wqk_f = w_attn[:, :2 * C].copy()
    wqk_f[:, :C] *= SCALE
    # [c, n] -> [kk, p, m, n'] -> [p, m, kk, n']
    wqk_pm = np.ascontiguousarray(
        wqk_f.reshape(KK, P, 2 * KK, P).transpose(1, 2, 0, 3)).astype(bf)

    bqk_f = b_attn[:2 * C].copy()
    bqk_f[:C] *= SCALE
    bqk_pm = np.ascontiguousarray(bqk_f.reshape(2 * KK, P).T).astype(np.float32)

    # column-chunk-major: [ct, p, kk, c]
    wv_cm = np.ascontiguousarray(
        w_attn[:, 2 * C:].reshape(KK, P, NCT, TQ).transpose(2, 1, 0, 3)
    ).astype(bf)
    wp_cm = np.ascontiguousarray(
        w_proj.reshape(KK, P, NCT, TQ).transpose(2, 1, 0, 3)).astype(bf)

    bv_bc = np.ascontiguousarray(
        np.broadcast_to(b_attn[2 * C:], (P, C))).astype(bf)
    bp_bc = np.ascontiguousarray(np.broadcast_to(b_proj, (P, C))).astype(bf)

    kk_i = np.arange(P)[:, None]
    qq_i = np.arange(P)[None, :]
    tri = (qq_i >= kk_i).astype(bf)
    ones_b = np.ones((P, P), dtype=bf)
    oinv_b = np.full((P, P), 1.0 / P, dtype=bf)

    common = dict(wqk_pm=wqk_pm, bqk_pm=bqk_pm, wv_cm=wv_cm, wp_cm=wp_cm,
                  bv_bc=bv_bc, bp_bc=bp_bc, tri=tri, ones_b=ones_b,
                  oinv_b=oinv_b)
    # [b, t, kk, p] -> [b, p, kk, t]
    xT_all = np.ascontiguousarray(
        x.reshape(B, T, KK, P).transpose(0, 3, 2, 1)).astype(bf)
    return [dict(xT_pm=np.ascontiguousarray(xT_all[i]), **common)
            for i in range(B)]


def run_spmd(inputs, trace=False, **kw):
    nc = _get_nc()
    in_maps = make_in_maps(inputs)
    return run_bass_kernel_spmd(nc, in_maps, list(range(N_CORES)),
                                trace=trace, **kw)


def kernel(**inputs):
    res = run_spmd(inputs, trace=False)
    y = np.stack([np.asarray(res.results[i]["y"]) for i in range(N_CORES)])
    return y.astype(np.float32)


if __name__ == "__main__":
    rng = np.random.default_rng(0)
    demo = {
        "x": rng.standard_normal((B, T, C)).astype(np.float32),
        "w_attn": (rng.standard_normal((C, 3 * C)) * 0.02).astype(np.float32),
        "b_attn": (rng.standard_normal(3 * C) * 0.02).astype(np.float32),
        "w_proj": (rng.standard_normal((C, C)) * 0.02).astype(np.float32),
        "b_proj": (rng.standard_normal(C) * 0.02).astype(np.float32),
    }
    out = kernel(**demo)
    print("out", out.shape, out.dtype, float(np.abs(out).max()))


# revision 82
# speedup vs baseline: 1.0024x; 1.0024x over previous
"""Causal self-attention (B=8, T=1024, C=2048, H=16) on 8 TRN2 NeuronCores.

Strategy: data-parallel over batch — core i computes the full attention block
for batch element i (weights replicated, no collectives).

Key optimizations (667us -> 591us -> ~533.5us at nominal clock):
  - x is transposed and cast to bf16 on the HOST: xT tiles DMA straight into
    SBUF, eliminating the on-device transpose phase (128 PE transposes +
    128 DVE copies + staging buffers, ~40us of low-PE-occupancy time)
  - B1's input feed is balanced across the three DMA-capable queues
    (sync/scalar/gpsimd) in consumption order with 0.25-2MB transfers
    (small DMAs only sustain ~100GB/s per queue; big ones ~370GB/s), so
    the first matmul issues ~11.5us in and B1 streams without stalls
  - head-0 k-weights preload into a persistent pool during B1: the normal
    per-head weight staging reuses the w_v SBUF space, and the pool-close
    barrier would gate that DMA on the LAST B1 matmul
  - B1 passes 1-3 run t-major so each PSUM bank closes early and its
    bias-add overlaps the remaining matmuls; pass 0 stays kk-major to match
    the wv column-0 stream (the B2C pools reuse these banks and the
    pool-close barrier waits for the last bias-add)
  - the drain's three psD PSUM banks empty via plain copies (bias already
    folded into PSUM) split across the ACT and DVE engines in parallel
  - per-head pipelining: each head's attention (softmax on ACT/DVE) is
    emitted interleaved with the NEXT head's q/k projection matmuls
  - fine-causal S/exp: S matmuls and exp cover only live columns
  - softmax denominators: bf16 tree-adds on the DVE, then cross-partition
    sums on the otherwise-idle gpsimd (partition_all_reduce) in the head
    loop; the latency-critical drain uses PE ones-matmuls instead
  - drain of the last head interleaves with phase-D column 0 (t=0..2) in
    the freed psB PSUM banks; the output bias for those tiles is folded
    into PSUM via a (1/128)-matmul against the broadcast bias so the final
    PSUM->SBUF drain is a plain ACT copy off the DVE critical path
  - phase D: w_proj column 0 preloaded during the last 4 heads (gpsimd);
    columns 1-3 stream in halves on queue pairs at phase-D start; all
    columns run t-major so bias-adds + stores spread out and PSUM banks
    recycle without column-boundary stalls; the final tile's bias+store is
    split in half across queues to start the end-of-kernel drain sooner

Per-core pipeline (Tile framework, all matmuls bf16 on the PE):
  B1) v = x @ Wv + bv in natural layout (8 PSUM banks, one per t-tile),
      consuming the xT / wv streams as they land
  B2+C) per head: k/q chunk matmuls (W-stationary, xT moving, bias on ACT)
     interleaved with the previous head's S^T = kT^T qT, exp on ACT,
     diagonal-triangle mask, denominator tree, PV accumulation, and the
     1/denom multiply -> attnT bf16
  D) y = attnT-stationary @ w_proj (moving, bf16) + bias, output f32.

Note: the chip occasionally runs the whole NEFF at a degraded 2.0GHz PE
clock (~640us instead of ~534us); this is a device power state, not a
property of the kernel schedule.
"""

import sys

if "/opt/trn_rl_repo" not in sys.path:
    sys.path.insert(0, "/opt/trn_rl_repo")

import numpy as np
import ml_dtypes

import concourse.bass as bass
import concourse.bass_isa as bass_isa
import concourse.mybir as mybir
import concourse.tile as tile
from concourse import bacc
from concourse.bass_utils import run_bass_kernel_spmd

B, T, C = 8, 1024, 2048
H, HD = 16, 128
N_CORES = 8
P = 128            # partition dim
TQ = 512           # moving-operand tile (q positions per matmul)
KK = C // P        # 16 contraction tiles over C
TT = T // P        # 8 tiles over T
NQ = T // TQ       # 2 q-tiles
NCT = C // TQ      # 4 column tiles over C
SCALE = 1.0 / float(np.sqrt(HD))

f32 = mybir.dt.float32
bf16 = mybir.dt.bfloat16
AFT = mybir.ActivationFunctionType

_NC_CACHE = None


def build_nc():
    nc = bacc.Bacc("TRN2", target_bir_lowering=False, debug=False,
                   num_devices=N_CORES)

    # x pre-transposed on host: xT_pm[p, kk, t] = x[t, kk*128+p], bf16
    xT_d = nc.declare_dram_parameter("xT_pm", [P, KK, T], bf16, isOutput=False)
    # q/k weight chunks, partition-major: wqk[p, m, kk, n] =
    # w_attn[kk*128+p, m*128+n] for m < 32 (q columns pre-scaled)
    wqk = nc.declare_dram_parameter("wqk_pm", [P, 2 * KK, KK, P], bf16,
                                    isOutput=False)
    bqk = nc.declare_dram_parameter("bqk_pm", [P, 2 * KK], f32, isOutput=False)
    # v / proj weights in column-chunk-major layout:
    #   wvc[ct, p, kk, c] = w_attn[kk*128+p, 2C + ct*512 + c]
    #   wpc[ct, p, kk, c] = w_proj[kk*128+p, ct*512 + c]
    wvc = nc.declare_dram_parameter("wv_cm", [NCT, P, KK, TQ], bf16,
                                    isOutput=False)
    wpc = nc.declare_dram_parameter("wp_cm", [NCT, P, KK, TQ], bf16,
                                    isOutput=False)
    bv = nc.declare_dram_parameter("bv_bc", [P, C], bf16, isOutput=False)
    bp = nc.declare_dram_parameter("bp_bc", [P, C], bf16, isOutput=False)
    tri_d = nc.declare_dram_parameter("tri", [P, P], bf16, isOutput=False)
    ones_b = nc.declare_dram_parameter("ones_b", [P, P], bf16, isOutput=False)
    oinv_b = nc.declare_dram_parameter("oinv_b", [P, P], bf16, isOutput=False)
    y = nc.declare_dram_parameter("y", [T, C], f32, isOutput=True)

    with tile.TileContext(nc) as tc:
        with tc.tile_pool(name="consts", bufs=1) as consts, \
             tc.tile_pool(name="resid", bufs=1) as resid, \
             tc.tile_pool(name="wp0", bufs=1) as wp0p, \
             tc.tile_pool(name="qkw0", bufs=1) as qkw0, \
             tc.tile_pool(name="ybuf", bufs=4) as ybuf:

            # ---- constants (DMAs issued inside the B1 feed below, ordered
            # by first use) ----
            bv_sb = consts.tile([P, C], bf16, tag="bv", name="bv")
            tri_sb = consts.tile([P, P], bf16, tag="tri", name="tri")
            ones_sb = consts.tile([P, P], bf16, tag="ones", name="ones")
            oinv_sb = consts.tile([P, P], bf16, tag="oinv", name="oinv")
            bqk_sb = consts.tile([P, 2 * KK], f32, tag="bqk", name="bqk")
            bp_sb = consts.tile([P, C], bf16, tag="bp", name="bp")

            # w_proj column 0, preloaded during the tail of B2+C
            wp0_sb = wp0p.tile([P, KK, TQ], bf16, tag="wp0", name="wp0")
            # head-0 q/k weights, preloaded during B1 on gpsimd so the first
            # B2+C matmul is not gated on a DMA that (because its SBUF space
            # is recycled from the w_v tiles) can only start after the last
            # B1 matmul finishes
            wtk0_sb = qkw0.tile([P, KK, P], bf16, tag="wtk0", name="wtk0")

            # ---- persistent intermediates (bf16) ----
            v = [resid.tile([P, C], bf16, tag=f"v{i}", name=f"v{i}") for i in range(TT)]
            attnT = [resid.tile([P, T], bf16, tag=f"attnT{i}", name=f"attnT{i}")
                     for i in range(H)]

            with tc.tile_pool(name="xT", bufs=1) as xTp:
                xT_sb = xTp.tile([P, KK, T], bf16, tag="xT", name="xT")
                # B1 feed balanced over the three DMA-capable queues
                # (~120GB/s per queue when all active; the DMA fabric
                # services queues roughly fairly, so each queue leads with
                # the earliest-consumed tiles):
                #   sync:   xT kk 0-7 in pairs, then wv col1 lo half
                #   scalar: wv col0 groups, then wv col1 hi half
                #   gpsimd: xT kk 8-15 in pairs, then bv, wv col2/3,
                #           consts, head-0 weights
                for lo, hi in ((0, 2), (2, 4), (4, 6), (6, 8)):
                    nc.sync.dma_start(out=xT_sb[:, lo:hi, :],
                                      in_=xT_d[:, lo:hi, :])
                for lo, hi in ((8, 10), (10, 12), (12, 14), (14, 16)):
                    nc.gpsimd.dma_start(out=xT_sb[:, lo:hi, :],
                                        in_=xT_d[:, lo:hi, :])

                with tc.tile_pool(name="wvp", bufs=1) as wvp:
                    wv_sb = [wvp.tile([P, KK, TQ], bf16, tag=f"wv{ct}",
                                      name=f"wv{ct}") for ct in range(NCT)]
                    for lo, hi in ((0, 2), (2, 4), (4, 8), (8, 12), (12, 16)):
                        nc.scalar.dma_start(out=wv_sb[0][:, lo:hi, :],
                                            in_=wvc[0, :, lo:hi, :])
                    nc.sync.dma_start(out=wv_sb[1][:, 0:8, :],
                                      in_=wvc[1, :, 0:8, :])
                    nc.scalar.dma_start(out=wv_sb[1][:, 8:16, :],
                                        in_=wvc[1, :, 8:16, :])
                    nc.gpsimd.dma_start(out=bv_sb, in_=bv[:])
                    nc.gpsimd.dma_start(out=wv_sb[2], in_=wvc[2])
                    nc.gpsimd.dma_start(out=wv_sb[3], in_=wvc[3])
                    # remaining consts + head-0 weights (gpsimd, after wv)
                    nc.gpsimd.dma_start(out=tri_sb, in_=tri_d[:])
                    nc.gpsimd.dma_start(out=ones_sb, in_=ones_b[:])
                    nc.gpsimd.dma_start(out=oinv_sb, in_=oinv_b[:])
                    nc.gpsimd.dma_start(out=bqk_sb, in_=bqk[:])
                    nc.gpsimd.dma_start(out=bp_sb, in_=bp[:])
                    nc.gpsimd.dma_start(out=wtk0_sb, in_=wqk[:, KK, :, :])

                    # ---- Phase B1: v = x @ Wv + bv, natural layout ----
                    # Pass 0 runs kk-major (matching the DMA arrival order of
                    # the streamed wv column-0 tiles); passes 1-3 run t-major
                    # so each psV bank closes early and its bias-add overlaps
                    # the remaining matmuls instead of bursting serialized on
                    # the DVE at the pass boundary (which gates the next
                    # pass's — and ultimately B2C's — bank reuse).
                    with tc.tile_pool(name="psV", bufs=8, space=bass.MemorySpace.PSUM) as psVp:
                        for ct in range(NCT):
                            psV = [psVp.tile([P, TQ], f32, tag="psV", name="psV")
                                   for _ in range(TT)]
                            if ct == 0:
                                for kk in range(KK):
                                    wvt = wv_sb[ct][:, kk, :]
                                    for t in range(TT):
                                        nc.tensor.matmul(
                                            psV[t],
                                            xT_sb[:, kk, t * P:(t + 1) * P],
                                            wvt,
                                            start=(kk == 0),
                                            stop=(kk == KK - 1))
                                for t in range(TT):
                                    nc.vector.tensor_add(
                                        v[t][:, ct * TQ:(ct + 1) * TQ], psV[t],
                                        bv_sb[:, ct * TQ:(ct + 1) * TQ])
                            else:
                                for t in range(TT):
                                    for kk in range(KK):
                                        nc.tensor.matmul(
                                            psV[t],
                                            xT_sb[:, kk, t * P:(t + 1) * P],
                                            wv_sb[ct][:, kk, :],
                                            start=(kk == 0),
                                            stop=(kk == KK - 1))
                                    nc.vector.tensor_add(
                                        v[t][:, ct * TQ:(ct + 1) * TQ],
                                        psV[t],
                                        bv_sb[:, ct * TQ:(ct + 1) * TQ])

                # ---- Merged phase B2+C: per head, the q/k projection
                # chunks (pure PE work) are interleaved with the PREVIOUS
                # head's attention so the softmax's ACT/DVE work and its
                # cross-engine latency hide completely under the projection
                # matmuls.
                #
                # Attention is fine-causal: S matmuls and exp cover only live
                # columns [kt*128, T); the permanently-masked columns of each
                # eS tile are zeroed ONCE and never written again, so the
                # denominator tree-adds can read full 512-wide slices. The
                # diagonal 128x128 triangle is zeroed by a 0/1 mask multiply.
                # PSUM: psB 3 + psS 3 + psO 2 = 8 banks; a matmul with
                # start=True clears its whole bank, so every accumulation
                # group owns a full bank.
                with tc.tile_pool(name="qkp", bufs=2) as qkp, \
                     tc.tile_pool(name="eSp", bufs=1) as eSp, \
                     tc.tile_pool(name="dsc", bufs=2) as dsc, \
                     tc.tile_pool(name="ctmp", bufs=2) as ctmp, \
                     tc.tile_pool(name="wst", bufs=2) as wst, \
                     tc.tile_pool(name="psB", bufs=3, space=bass.MemorySpace.PSUM) as psB, \
                     tc.tile_pool(name="psS", bufs=3, space=bass.MemorySpace.PSUM) as psS, \
                     tc.tile_pool(name="psO", bufs=2, space=bass.MemorySpace.PSUM) as psO:
                    eSab = [[eSp.tile([P, T], bf16, tag=f"eS{s}_{kt}",
                                      name=f"eS{s}_{kt}")
                             for kt in range(TT)] for s in range(2)]
                    for s in range(2):
                        for kt in range(1, TT):
                            nc.vector.memset(eSab[s][kt][:, 0:kt * P], 0)

                    tri = tri_sb  # [128,128] q>=k triangle
                    st = [dict() for _ in range(H)]

                    def emit_S(ph, kt):
                        pool, ptag = psS, "psS"
                        q0 = kt * P
                        eS = eSab[ph % 2]
                        kblk = st[ph]["kT"][:, kt * P:(kt + 1) * P]
                        qTt = st[ph]["qT"]
                        if kt < 4:
                            pa = pool.tile([P, TQ], f32, tag=ptag, name="pssa")
                            nc.tensor.matmul(pa[:, q0:TQ], kblk,
                                             qTt[:, q0:TQ],
                                             start=True, stop=True)
                            pb = pool.tile([P, TQ], f32, tag=ptag, name="pssb")
                            nc.tensor.matmul(pb, kblk, qTt[:, TQ:T],
                                             start=True, stop=True)
                            nc.scalar.activation(out=eS[kt][:, q0:TQ],
                                                 in_=pa[:, q0:TQ],
                                                 func=AFT.Exp)
                            nc.scalar.activation(out=eS[kt][:, TQ:T], in_=pb,
                                                 func=AFT.Exp)
                        else:
                            pb = pool.tile([P, TQ], f32, tag=ptag, name="pssb")
                            nc.tensor.matmul(pb[:, q0 - TQ:TQ], kblk,
                                             qTt[:, q0:T],
                                             start=True, stop=True)
                            nc.scalar.activation(out=eS[kt][:, q0:T],
                                                 in_=pb[:, q0 - TQ:TQ],
                                                 func=AFT.Exp)
                        nc.vector.tensor_mul(eS[kt][:, q0:q0 + P],
                                             eS[kt][:, q0:q0 + P], tri)

                    def emit_tree_qt1(ph):
                        eS = eSab[ph % 2]
                        t1 = dsc.tile([P, TQ], bf16, tag="t1", name="t1")
                        t2 = dsc.tile([P, TQ], bf16, tag="t2", name="t2")
                        t3 = dsc.tile([P, TQ], bf16, tag="t3", name="t3")
                        t4 = dsc.tile([P, TQ], bf16, tag="t4", name="t4")
                        nc.vector.tensor_add(t1, eS[0][:, TQ:T], eS[1][:, TQ:T])
                        nc.vector.tensor_add(t2, eS[2][:, TQ:T], eS[3][:, TQ:T])
                        nc.vector.tensor_add(t3, eS[4][:, TQ:T], eS[5][:, TQ:T])
                        nc.vector.tensor_add(t4, eS[6][:, TQ:T], eS[7][:, TQ:T])
                        nc.vector.tensor_add(t1, t1, t2)
                        nc.vector.tensor_add(t3, t3, t4)
                        nc.vector.tensor_add(t1, t1, t3)
                        st[ph]["t1"] = t1

                    def emit_tree_qt0(ph):
                        eS = eSab[ph % 2]
                        u1 = dsc.tile([P, TQ], bf16, tag="t5", name="u1")
                        u2 = dsc.tile([P, TQ], bf16, tag="t6", name="u2")
                        nc.vector.tensor_add(u1, eS[0][:, 0:TQ], eS[1][:, 0:TQ])
                        nc.vector.tensor_add(u2, eS[2][:, 0:TQ], eS[3][:, 0:TQ])
                        nc.vector.tensor_add(u1, u1, u2)
                        st[ph]["u1"] = u1

                    def emit_denoms(ph, on_pe=False):
                        # In the pipelined head loop the cross-partition sums
                        # run on the (idle) gpsimd engine — frees the PE of
                        # 2x512 ones-matmul columns per head. gpsimd takes
                        # ~3.6us per reduce though, so the latency-critical
                        # DRAIN uses the PE ones-matmul (0.2us each) instead.
                        if on_pe:
                            psd1 = psS.tile([P, TQ], f32, tag="psS", name="psd1")
                            nc.tensor.matmul(psd1, ones_sb, st[ph]["t1"],
                                             start=True, stop=True)
                            psd0 = psS.tile([P, TQ], f32, tag="psS", name="psd0")
                            nc.tensor.matmul(psd0, ones_sb, st[ph]["u1"],
                                             start=True, stop=True)
                            st[ph]["psd0"], st[ph]["psd1"] = psd0, psd1
                            return
                        d1 = ctmp.tile([P, TQ], f32, tag="d1", name="d1")
                        nc.gpsimd.partition_all_reduce(
                            d1, st[ph]["t1"], channels=P,
                            reduce_op=bass_isa.ReduceOp.add)
                        d0 = ctmp.tile([P, TQ], f32, tag="d0", name="d0")
                        nc.gpsimd.partition_all_reduce(
                            d0, st[ph]["u1"], channels=P,
                            reduce_op=bass_isa.ReduceOp.add)
                        st[ph]["psd0"], st[ph]["psd1"] = d0, d1

                    def emit_PV(ph, kt):
                        eS = eSab[ph % 2]
                        if kt == 0:
                            st[ph]["pso0"] = psO.tile([P, TQ], f32, tag="psO",
                                                      name="pso0")
                            st[ph]["pso1"] = psO.tile([P, TQ], f32, tag="psO",
                                                      name="pso1")
                        lhsT = v[kt][:, ph * P:(ph + 1) * P]
                        # start=True must span the whole bank (it clears it);
                        # accumulating matmuls shrink to the live columns
                        if kt == 0:
                            nc.tensor.matmul(
                                st[ph]["pso0"], lhsT, eS[0][:, 0:TQ],
                                start=True, stop=False)
                            nc.tensor.matmul(
                                st[ph]["pso1"], lhsT, eS[0][:, TQ:T],
                                start=True, stop=False)
                        else:
                            q0 = kt * P
                            if kt < 4:
                                nc.tensor.matmul(
                                    st[ph]["pso0"][:, q0:TQ], lhsT,
                                    eS[kt][:, q0:TQ],
                                    start=False, stop=(kt == 3))
                                nc.tensor.matmul(
                                    st[ph]["pso1"], lhsT, eS[kt][:, TQ:T],
                                    start=False, stop=False)
                            else:
                                lo = max(q0, TQ)
                                nc.tensor.matmul(
                                    st[ph]["pso1"][:, lo - TQ:TQ], lhsT,
                                    eS[kt][:, lo:T],
                                    start=False, stop=(kt == TT - 1))

                    def emit_div(ph):
                        # ~18-bit reciprocal; denominators in [1, ~2e5]
                        rec1 = ctmp.tile([P, TQ], f32, tag="rec", name="rec1")
                        nc.vector.reciprocal_approx_fast(out=rec1,
                                                         in_=st[ph]["psd1"])
                        nc.vector.tensor_mul(attnT[ph][:, TQ:T],
                                             st[ph]["pso1"], rec1)
                        rec0 = ctmp.tile([P, TQ], f32, tag="rec", name="rec0")
                        nc.vector.reciprocal_approx_fast(out=rec0,
                                                         in_=st[ph]["psd0"])
                        nc.vector.tensor_mul(attnT[ph][:, 0:TQ],
                                             st[ph]["pso0"], rec0)
                        st[ph].clear()

                    for i in range(H):
                        ph = i - 1
                        # --- k chunk of head i ---
                        # wqk DMAs issue from the (idle) sync queue — the
                        # scalar sequencer is busy with exps/identities
                        if i == 0:
                            wtk = wtk0_sb
                        else:
                            wtk = wst.tile([P, KK, P], bf16, tag="wt", name="wtk")
                            nc.sync.dma_start(out=wtk, in_=wqk[:, KK + i, :, :])
                        kTt = qkp.tile([P, T], bf16, tag="kT", name="kTt")
                        psk = [psB.tile([P, TQ], f32, tag="psB", name="psB")
                               for _ in range(NQ)]
                        for kk in range(KK):
                            for qt in range(NQ):
                                nc.tensor.matmul(
                                    psk[qt], wtk[:, kk, :],
                                    xT_sb[:, kk, qt * TQ:(qt + 1) * TQ],
                                    start=(kk == 0), stop=(kk == KK - 1))
                            if ph >= 0 and kk < TT:
                                emit_S(ph, kk)
                        for qt in range(NQ):
                            nc.scalar.activation(
                                out=kTt[:, qt * TQ:(qt + 1) * TQ], in_=psk[qt],
                                func=AFT.Identity,
                                bias=bqk_sb[:, KK + i:KK + i + 1])
                        # --- q chunk of head i ---
                        wtq = wst.tile([P, KK, P], bf16, tag="wt", name="wtq")
                        nc.gpsimd.dma_start(out=wtq, in_=wqk[:, i, :, :])
                        # preload w_proj column 0 during the last 4 heads
                        # (gpsimd queue is otherwise idle by then)
                        if i >= H - 4:
                            q4 = i - (H - 4)
                            nc.gpsimd.dma_start(
                                out=wp0_sb[:, q4 * 4:(q4 + 1) * 4, :],
                                in_=wpc[0, :, q4 * 4:(q4 + 1) * 4, :])
                        qTt = qkp.tile([P, T], bf16, tag="qT", name="qTt")
                        psq = [psB.tile([P, TQ], f32, tag="psB", name="psB")
                               for _ in range(NQ)]
                        for kk in range(KK):
                            for qt in range(NQ):
                                nc.tensor.matmul(
                                    psq[qt], wtq[:, kk, :],
                                    xT_sb[:, kk, qt * TQ:(qt + 1) * TQ],
                                    start=(kk == 0), stop=(kk == KK - 1))
                            if ph >= 0:
                                if kk == 0:
                                    emit_tree_qt1(ph)
                                elif kk == 2:
                                    emit_tree_qt0(ph)
                                elif kk == 3:
                                    emit_denoms(ph)
                                if kk % 2 == 1:
                                    emit_PV(ph, kk // 2)
                        for qt in range(NQ):
                            nc.scalar.activation(
                                out=qTt[:, qt * TQ:(qt + 1) * TQ], in_=psq[qt],
                                func=AFT.Identity, bias=bqk_sb[:, i:i + 1])
                        if ph >= 0:
                            emit_div(ph)
                        st[i]["kT"], st[i]["qT"] = kTt, qTt
                    # --- drain: attention of the last head, interleaved with
                    # phase-D column 0, t-tiles 0..2 (using the 3 freed psB
                    # banks + the preloaded wp0) so the PE never idles while
                    # the softmax's ACT/DVE chain runs ---
                    ph = H - 1
                    for kt in range(TT):
                        emit_S(ph, kt)
                    psD = [psB.tile([P, TQ], f32, tag="psB", name=f"psD{t}")
                           for t in range(3)]

                    def dcol0(lo, hi):
                        for kk in range(lo, hi):
                            wpt = wp0_sb[:, kk, :]
                            for t in range(3):
                                nc.tensor.matmul(
                                    psD[t], attnT[kk][:, t * P:(t + 1) * P],
                                    wpt, start=(kk == 0), stop=False)

                    dcol0(0, 5)
                    # fold the output bias into the PSUM accumulation
                    # (oinv^T @ bp_bc = broadcast bias row) so the final
                    # PSUM->SBUF drain is a plain copy the ACT engine can do
                    for t in range(3):
                        nc.tensor.matmul(psD[t], oinv_sb, bp_sb[:, 0:TQ],
                                         start=False, stop=False)
                    emit_tree_qt1(ph)
                    emit_tree_qt0(ph)
                    dcol0(5, 9)
                    emit_denoms(ph, on_pe=True)
                    # reciprocals early (they only need the denominators) so
                    # the post-PV critical chain is just the two multiplies;
                    # the [TQ:T] half is not needed by the kk=15 psD matmuls
                    # so it runs on gpsimd off the critical path
                    rec1 = ctmp.tile([P, TQ], f32, tag="rec", name="rec1d")
                    nc.vector.reciprocal_approx_fast(out=rec1,
                                                     in_=st[ph]["psd1"])
                    rec0 = ctmp.tile([P, TQ], f32, tag="rec", name="rec0d")
                    nc.vector.reciprocal_approx_fast(out=rec0,
                                                     in_=st[ph]["psd0"])
                    for kt in range(TT):
                        emit_PV(ph, kt)
                    dcol0(9, 15)
                    # [0:TQ] first — the kk=15 psD matmuls only need that half
                    nc.vector.tensor_mul(attnT[ph][:, 0:TQ],
                                         st[ph]["pso0"], rec0)
                    nc.vector.tensor_mul(attnT[ph][:, TQ:T],
                                         st[ph]["pso1"], rec1)
                    st[ph].clear()
                    for t in range(3):
                        nc.tensor.matmul(
                            psD[t], attnT[H - 1][:, t * P:(t + 1) * P],
                            wp0_sb[:, KK - 1, :], start=False, stop=True)
                    # drain the three psD banks on two engines in parallel
                    # (plain copies — the bias is already in PSUM); the B2C
                    # pool close, and with it phase D, waits on the last one
                    for t in range(3):
                        y_sb = ybuf.tile([P, TQ], f32, tag="y_sb", name="y_sb")
                        if t == 1:
                            nc.vector.tensor_copy(y_sb, psD[t])
                        else:
                            nc.scalar.activation(out=y_sb, in_=psD[t],
                                                 func=AFT.Identity)
                        deng = nc.sync if t % 2 == 0 else nc.scalar
                        deng.dma_start(out=y[t * P:(t + 1) * P, 0:TQ],
                                       in_=y_sb)

            # ---- Phase D (rest): output projection ----
            # Column 0 t=0..2 already computed during the drain. Columns 1-3
            # stream as single 2MB DMAs issued up-front on three different
            # queues so they always arrive ahead of consumption. The last
            # column runs t-major so the final bias-adds + stores spread
            # across its whole span instead of bunching at the end.
            with tc.tile_pool(name="wps", bufs=1) as wps:
                  with tc.tile_pool(name="psY", bufs=8, space=bass.MemorySpace.PSUM) as psYp:
                      wp_sb = [wp0_sb] + [
                          wps.tile([P, KK, TQ], bf16, tag=f"wp{ct}",
                                   name=f"wp{ct}") for ct in range(1, NCT)]
                      # each column split across two queues so col1 lands
                      # before its first matmul needs it (columns are
                      # consumed ~14us apart; a full 6MB three-way race
                      # delivers col1 too late)
                      for ct, engs in ((1, (nc.gpsimd, nc.sync)),
                                       (2, (nc.sync, nc.scalar)),
                                       (3, (nc.gpsimd, nc.scalar))):
                          engs[0].dma_start(out=wp_sb[ct][:, 0:8, :],
                                            in_=wpc[ct, :, 0:8, :])
                          engs[1].dma_start(out=wp_sb[ct][:, 8:16, :],
                                            in_=wpc[ct, :, 8:16, :])
                      # Never-used dummy allocs: skip the PSUM banks still
                      # owned by the drain's psD tiles (their bias-adds are
                      # serialized on the DVE) so the first matmuls land on
                      # banks the exps freed earlier.
                      for _ in range(3):
                          psYp.tile([P, TQ], f32, tag="psY", name="psYdummy")
                      # all columns t-major: each t-tile's accumulation
                      # closes early, so its bias-add + store overlap the
                      # next t-tile's matmuls instead of bursting at the
                      # column end and stalling the next column's banks
                      for ct in range(NCT):
                          ts = range(3, TT) if ct == 0 else range(TT)
                          for t in ts:
                              psY = psYp.tile([P, TQ], f32, tag="psY",
                                              name="psY")
                              for kk in range(KK):
                                  nc.tensor.matmul(
                                      psY, attnT[kk][:, t * P:(t + 1) * P],
                                      wp_sb[ct][:, kk, :],
                                      start=(kk == 0), stop=(kk == KK - 1))
                              y_sb = ybuf.tile([P, TQ], f32, tag="y_sb",
                                               name="y_sb")
                              c0 = ct * TQ
                              if ct == NCT - 1 and t == TT - 1:
                                  # final tile: bias+store in halves so the
                                  # kernel-end drain starts sooner
                                  h = TQ // 2
                                  nc.vector.tensor_add(
                                      y_sb[:, 0:h], psY[:, 0:h],
                                      bp_sb[:, c0:c0 + h])
                                  nc.sync.dma_start(
                                      out=y[t * P:(t + 1) * P, c0:c0 + h],
                                      in_=y_sb[:, 0:h])
                                  nc.vector.tensor_add(
                                      y_sb[:, h:TQ], psY[:, h:TQ],
                                      bp_sb[:, c0 + h:c0 + TQ])
                                  nc.scalar.dma_start(
                                      out=y[t * P:(t + 1) * P,
                                            c0 + h:c0 + TQ],
                                      in_=y_sb[:, h:TQ])
                              else:
                                  nc.vector.tensor_add(
                                      y_sb, psY, bp_sb[:, c0:c0 + TQ])
                                  deng = nc.sync if t % 2 == 0 else nc.scalar
                                  deng.dma_start(
                                      out=y[t * P:(t + 1) * P, c0:c0 + TQ],
                                      in_=y_sb)

    nc.compile()
    return nc


def _get_nc():
    global _NC_CACHE
    if _NC_CACHE is None:
        _NC_CACHE = build_nc()
    return _NC_CACHE


def make_in_maps(inputs):
    x = np.asarray(inputs["x"], dtype=np.float32)
    w_attn = np.asarray(inputs["w_attn"], dtype=np.float32)
    b_attn = np.asarray(inputs["b_attn"], dtype=np.float32)
    w_proj = np.asarray(inputs["w_proj"], dtype=np.float32)
    b_proj = np.asarray(inputs["b_proj"], dtype=np.float32)

    bf = ml_dtypes.bfloat16

    # q/k weights, scale folded into q: [P, 32, KK, P] partition-major
    wqk_f = w_attn[:, :2 * C].copy()
    wqk_f[:, :C] *= SCALE
    # [c, n] -> [kk, p, m, n'] -> [p, m, kk, n']
    wqk_pm = np.ascontiguousarray(
        wqk_f.reshape(KK, P, 2 * KK, P).transpose(1, 2, 0, 3)).astype(bf)

    bqk_f = b_attn[:2 * C].copy()
    bqk_f[:C] *= SCALE
    bqk_pm = np.ascontiguousarray(bqk_f.reshape(2 * KK, P).T).astype(np.float32)

    # column-chunk-major: [ct, p, kk, c]
    wv_cm = np.ascontiguousarray(
        w_attn[:, 2 * C:].reshape(KK, P, NCT, TQ).transpose(2, 1, 0, 3)
    ).astype(bf)
    wp_cm = np.ascontiguousarray(
        w_proj.reshape(KK, P, NCT, TQ).transpose(2, 1, 0, 3)).astype(bf)

    bv_bc = np.ascontiguousarray(
        np.broadcast_to(b_attn[2 * C:], (P, C))).astype(bf)
    bp_bc = np.ascontiguousarray(np.broadcast_to(b_proj, (P, C))).astype(bf)

    kk_i = np.arange(P)[:, None]
    qq_i = np.arange(P)[None, :]
    tri = (qq_i >= kk_i).astype(bf)
    ones_b = np.ones((P, P), dtype=bf)
    oinv_b = np.full((P, P), 1.0 / P, dtype=bf)

    common = dict(wqk_pm=wqk_pm, bqk_pm=bqk_pm, wv_cm=wv_cm, wp_cm=wp_cm,
                  bv_bc=bv_bc, bp_bc=bp_bc, tri=tri, ones_b=ones_b,
                  oinv_b=oinv_b)
    # [b, t, kk, p] -> [b, p, kk, t]
    xT_all = np.ascontiguousarray(
        x.reshape(B, T, KK, P).transpose(0, 3, 2, 1)).astype(bf)
    return [dict(xT_pm=np.ascontiguousarray(xT_all[i]), **common)
            for i in range(B)]


def run_spmd(inputs, trace=False, **kw):
    nc = _get_nc()
    in_maps = make_in_maps(inputs)
    return run_bass_kernel_spmd(nc, in_maps, list(range(N_CORES)),
                                trace=trace, **kw)


def kernel(**inputs):
    res = run_spmd(inputs, trace=False)
    y = np.stack([np.asarray(res.results[i]["y"]) for i in range(N_CORES)])
    return y.astype(np.float32)


if __name__ == "__main__":
    rng = np.random.default_rng(0)
    demo = {
        "x": rng.standard_normal((B, T, C)).astype(np.float32),
        "w_attn": (rng.standard_normal((C, 3 * C)) * 0.02).astype(np.float32),
        "b_attn": (rng.standard_normal(3 * C) * 0.02).astype(np.float32),
        "w_proj": (rng.standard_normal((C, C)) * 0.02).astype(np.float32),
        "b_proj": (rng.standard_normal(C) * 0.02).astype(np.float32),
    }
    out = kernel(**demo)
    print("out", out.shape, out.dtype, float(np.abs(out).max()))
